# revision 6
# baseline (speedup 1.0000x reference)
"""Adaptive-softmax (AdaSoftmaxGenerator) distributed Bass kernel for 8 trn2 cores.

Strategy: vocab-parallel. Each core owns a slice of every softmax group:
  head: 2500 of 20000 direct cols (+2 replicated cluster cols, +58 pad) = 2560
  tail1: 8500 of 68000 (+204 pad) = 8704
  tail0: 5000 of 40000 (+120 pad) = 5120
Total 16384 = 32 col-tiles of 512. Per-core device columns are ordered
[head | tail1 | tail0] (processing order: head stats must come first since
its logZ feeds both tail offsets; tail0 last minimizes the exposed tail).

Per (col-tile, batch-chunk): bf16 matmul x@W^T into PSUM f32, DVE adds the
per-column bias and writes a bf16 "spill" tile to the output DRAM, ACT
computes exp with a fused per-row accumulation (no max subtraction needed:
logits are O(10), exp is safe in f32). Per-group row-sums are AllReduced
across the 8 cores (tiny 4KB payload), then a second pass applies the
per-row offset (-logZ + cluster log-prob) in-place on the output.

The 2 cluster columns are computed identically on all 8 cores inside the
head region; the AllReduce over-counts them 8x, corrected by subtracting
7*exp(c) post-reduce (bit-identical across cores, so exact).
"""

import sys
import types

sys.path.insert(0, "/opt/trn_rl_repo")

import numpy as np
import ml_dtypes

import concourse.bass as bass  # noqa: F401
import concourse.mybir as mybir
import concourse.tile as tile
from concourse import bacc
from concourse.bass_utils import run_bass_kernel_spmd
from concourse.tile_rust import add_dep_helper

F32 = mybir.dt.float32
BF16 = mybir.dt.bfloat16
AF = mybir.ActivationFunctionType
ALU = mybir.AluOpType
AX = mybir.AxisListType

NCORES = 8
B = 1024
D = 1024
P = 128
NT = 512  # col-tile width
H_OWN, T1_OWN, T0_OWN = 2500, 8500, 5000
HEAD_COLS, T1_COLS, T0_COLS = 2560, 8704, 5120  # padded per-core regions
NCOLS = HEAD_COLS + T1_COLS + T0_COLS  # 16384
GROUPS = [  # (j0, ntiles) in processing order: head, tail1, tail0
    (0, HEAD_COLS // NT),  # 0..4
    (HEAD_COLS // NT, T1_COLS // NT),  # 5..21
    ((HEAD_COLS + T1_COLS) // NT, T0_COLS // NT),  # 22..31
]
CL_TILE = 4  # cluster cols 2500,2501 live in head tile 4 at offsets 452,453
CL_OFF = 2500 - 4 * NT  # 452
PAD_BIAS = -10000.0

_cached_nc = None


def build(phase2_at_end=False, skip_phase2=False, zero_off=False):
    nc = bacc.Bacc(None, target_bir_lowering=False, debug=False)

    xt_d = nc.declare_dram_parameter("xt", [D, B], BF16, isOutput=False)
    wt_d = nc.declare_dram_parameter("wt", [D, NCOLS], BF16, isOutput=False)
    bias_d = nc.declare_dram_parameter("bias", [P, NCOLS], BF16, isOutput=False)
    out_d = nc.declare_dram_parameter("out", [B, NCOLS], BF16, isOutput=True)

    st_in = [nc.dram_tensor(f"st_in{g}", [P, 8], F32) for g in range(3)]
    st_out = [
        nc.dram_tensor(f"st_out{g}", [P, 8], F32, addr_space="Shared")
        for g in range(3)
    ]

    xt_r = xt_d[:, :].rearrange("(k p) b -> p k b", p=P)
    wt_r = wt_d[:, :].rearrange("(k p) c -> p k c", p=P)

    with tile.TileContext(nc) as tc:
        with (
            tc.tile_pool(name="xt", bufs=1) as xt_pool,
            tc.tile_pool(name="w", bufs=4) as w_pool,
            tc.tile_pool(name="bias", bufs=4) as b_pool,
            tc.tile_pool(name="ps", bufs=4, space="PSUM") as ps_pool,
            tc.tile_pool(name="spill", bufs=6) as sp_pool,
            tc.tile_pool(name="exp", bufs=4) as ex_pool,
            tc.tile_pool(name="st", bufs=1) as st_pool,
            tc.tile_pool(name="rmw", bufs=8) as rw_pool,
        ):
            xt_sb = xt_pool.tile([P, 8, B], BF16, tag="xt")
            nc.sync.dma_start(out=xt_sb[:, :, :], in_=xt_r)

            c0_sb = st_pool.tile([P, 8], F32, tag="c0")
            c1_sb = st_pool.tile([P, 8], F32, tag="c1")
            lzh = st_pool.tile([P, 8], F32, tag="lzh")
            spill_dmas = {}
            deferred_p2 = []

            for g, (j0, nt_g) in enumerate(GROUPS):
                sc = st_pool.tile([P, 8 * nt_g], F32, tag=f"sc{g}")
                for jj in range(nt_g):
                    j = j0 + jj
                    w_sb = w_pool.tile([P, 8, NT], BF16, tag="w")
                    nc.sync.dma_start(
                        out=w_sb[:, :, :], in_=wt_r[:, :, j * NT : (j + 1) * NT]
                    )
                    b_sb = b_pool.tile([P, NT], BF16, tag="bias")
                    nc.sync.dma_start(
                        out=b_sb[:, :], in_=bias_d[:, j * NT : (j + 1) * NT]
                    )
                    for bi in range(8):
                        psum = ps_pool.tile([P, NT], F32, tag="ps")
                        for k in range(8):
                            nc.tensor.matmul(
                                psum[:, :],
                                xt_sb[:, k, bi * P : (bi + 1) * P],
                                w_sb[:, k, :],
                                start=(k == 0),
                                stop=(k == 7),
                            )
                        spill = sp_pool.tile([P, NT], BF16, tag="spill")
                        nc.vector.tensor_tensor(
                            spill[:, :], psum[:, :], b_sb[:, :], op=ALU.add
                        )
                        exp_t = ex_pool.tile([P, NT], F32, tag="exp")
                        slot = bi * nt_g + jj
                        nc.scalar.activation(
                            exp_t[:, :],
                            spill[:, :],
                            AF.Exp,
                            accum_out=sc[:, slot : slot + 1],
                        )
                        dma = nc.sync.dma_start(
                            out=out_d[bi * P : (bi + 1) * P, j * NT : (j + 1) * NT],
                            in_=spill[:, :],
                        )
                        spill_dmas[(bi, j)] = dma
                        if g == 0 and jj == CL_TILE:
                            nc.vector.tensor_copy(
                                c0_sb[:, bi : bi + 1],
                                spill[:, CL_OFF : CL_OFF + 1],
                            )
                            nc.vector.tensor_copy(
                                c1_sb[:, bi : bi + 1],
                                spill[:, CL_OFF + 1 : CL_OFF + 2],
                            )

                # group row-sums -> AllReduce -> per-row offset
                st_sb = st_pool.tile([P, 8], F32, tag=f"st{g}")
                for bi in range(8):
                    nc.vector.tensor_reduce(
                        st_sb[:, bi : bi + 1],
                        sc[:, bi * nt_g : (bi + 1) * nt_g],
                        axis=AX.X,
                        op=ALU.add,
                    )
                nc.sync.dma_start(out=st_in[g][:, :], in_=st_sb[:, :])
                nc.gpsimd.collective_compute(
                    "AllReduce",
                    ALU.add,
                    replica_groups=[list(range(NCORES))],
                    ins=[st_in[g][:, :]],
                    outs=[st_out[g][:, :]],
                )
                s_ar = st_pool.tile([P, 8], F32, tag=f"sar{g}")
                nc.sync.dma_start(out=s_ar[:, :], in_=st_out[g][:, :])

                off = st_pool.tile([P, 8], F32, tag=f"off{g}")
                if g == 0:
                    # true head sum = AR sum - 7 * (exp(c0) + exp(c1))
                    e0 = st_pool.tile([P, 8], F32, tag="e0")
                    e1 = st_pool.tile([P, 8], F32, tag="e1")
                    nc.scalar.activation(e0[:, :], c0_sb[:, :], AF.Exp)
                    nc.scalar.activation(e1[:, :], c1_sb[:, :], AF.Exp)
                    ee = st_pool.tile([P, 8], F32, tag="ee")
                    nc.vector.tensor_add(ee[:, :], e0[:, :], e1[:, :])
                    nc.vector.tensor_scalar_mul(ee[:, :], ee[:, :], -7.0)
                    strue = st_pool.tile([P, 8], F32, tag="strue")
                    nc.vector.tensor_add(strue[:, :], s_ar[:, :], ee[:, :])
                    nc.scalar.activation(lzh[:, :], strue[:, :], AF.Ln)
                    nc.vector.tensor_scalar_mul(off[:, :], lzh[:, :], -1.0)
                else:
                    lzg = st_pool.tile([P, 8], F32, tag=f"lzg{g}")
                    nc.scalar.activation(lzg[:, :], s_ar[:, :], AF.Ln)
                    csrc = c1_sb if g == 1 else c0_sb
                    nc.vector.tensor_sub(off[:, :], csrc[:, :], lzh[:, :])
                    nc.vector.tensor_sub(off[:, :], off[:, :], lzg[:, :])

                if zero_off:
                    nc.vector.memset(off[:, :], 0.0)

                # phase 2: in-place RMW of this group's output region
                def emit_phase2(j0=j0, nt_g=nt_g, off=off):
                    for jj in range(nt_g):
                        j = j0 + jj
                        for bi in range(8):
                            rmw = rw_pool.tile([P, NT], BF16, tag="rmw")
                            rd = nc.sync.dma_start(
                                out=rmw[:, :],
                                in_=out_d[
                                    bi * P : (bi + 1) * P, j * NT : (j + 1) * NT
                                ],
                            )
                            add_dep_helper(
                                rd.ins,
                                spill_dmas[(bi, j)].ins,
                                reason="rmw-after-spill",
                            )
                            nc.vector.tensor_scalar_add(
                                rmw[:, :], rmw[:, :], off[:, bi : bi + 1]
                            )
                            nc.sync.dma_start(
                                out=out_d[
                                    bi * P : (bi + 1) * P, j * NT : (j + 1) * NT
                                ],
                                in_=rmw[:, :],
                            )

                if skip_phase2:
                    pass
                elif phase2_at_end:
                    deferred_p2.append(emit_phase2)
                else:
                    emit_phase2()

            for fn in deferred_p2:
                fn()

    nc.compile()
    return nc


def get_nc():
    global _cached_nc
    if _cached_nc is None:
        _cached_nc = build()
    return _cached_nc


def make_in_maps(x, head_w, head_b, tail0_w, tail0_b, tail1_w, tail1_b):
    bf = ml_dtypes.bfloat16
    x = np.asarray(x, np.float32)
    xt = np.ascontiguousarray(x.T).astype(bf)
    in_maps = []
    for c in range(NCORES):
        w_parts = [
            np.asarray(head_w[c * H_OWN : (c + 1) * H_OWN], np.float32),
            np.asarray(head_w[20000:20002], np.float32),
            np.zeros((HEAD_COLS - H_OWN - 2, D), np.float32),
            np.asarray(tail1_w[c * T1_OWN : (c + 1) * T1_OWN], np.float32),
            np.zeros((T1_COLS - T1_OWN, D), np.float32),
            np.asarray(tail0_w[c * T0_OWN : (c + 1) * T0_OWN], np.float32),
            np.zeros((T0_COLS - T0_OWN, D), np.float32),
        ]
        w = np.concatenate(w_parts, axis=0)  # [NCOLS, D]
        wt = np.ascontiguousarray(w.T).astype(bf)  # [D, NCOLS]
        b_parts = [
            np.asarray(head_b[c * H_OWN : (c + 1) * H_OWN], np.float32),
            np.asarray(head_b[20000:20002], np.float32),
            np.full(HEAD_COLS - H_OWN - 2, PAD_BIAS, np.float32),
            np.asarray(tail1_b[c * T1_OWN : (c + 1) * T1_OWN], np.float32),
            np.full(T1_COLS - T1_OWN, PAD_BIAS, np.float32),
            np.asarray(tail0_b[c * T0_OWN : (c + 1) * T0_OWN], np.float32),
            np.full(T0_COLS - T0_OWN, PAD_BIAS, np.float32),
        ]
        bias = np.concatenate(b_parts).astype(bf)  # [NCOLS]
        bias_bc = np.ascontiguousarray(np.broadcast_to(bias, (P, NCOLS)))
        in_maps.append({"xt": xt, "wt": wt, "bias": bias_bc})
    return in_maps


def assemble(results):
    prob = np.empty((B, 128000), np.float32)
    for c in range(NCORES):
        o = results[c]["out"].astype(np.float32)
        prob[:, c * H_OWN : (c + 1) * H_OWN] = o[:, :H_OWN]
        prob[:, 60000 + c * T1_OWN : 60000 + (c + 1) * T1_OWN] = o[
            :, HEAD_COLS : HEAD_COLS + T1_OWN
        ]
        prob[:, 20000 + c * T0_OWN : 20000 + (c + 1) * T0_OWN] = o[
            :, HEAD_COLS + T1_COLS : HEAD_COLS + T1_COLS + T0_OWN
        ]
    return prob


def kernel(x, head_w, head_b, tail0_w, tail0_b, tail1_w, tail1_b):
    in_maps = make_in_maps(x, head_w, head_b, tail0_w, tail0_b, tail1_w, tail1_b)
    nc = get_nc()
    res = run_bass_kernel_spmd(nc, in_maps, core_ids=list(range(NCORES)))
    return assemble(res.results)


def run_traced(inputs):
    """Run with NTFF profiling; returns (prob, BassKernelResults)."""
    _hooks = types.ModuleType("antenv.axon_hooks")
    _hooks._hook = None
    _hooks.set_axon_ntff_profile_hook = lambda h: setattr(_hooks, "_hook", h)
    _hooks.get_axon_ntff_profile_hook = lambda: _hooks._hook
    sys.modules["antenv.axon_hooks"] = _hooks
    import antenv

    antenv.axon_hooks = _hooks
    from trn_agent_boot.trn_boot import _ntff_profile_via_ctypes

    _hooks.set_axon_ntff_profile_hook(
        _ntff_profile_via_ctypes("/opt/axon/libaxon_pjrt.so")
    )
    from concourse import bass_utils as _bu

    _bu.upload_artifacts = lambda tmpdir: tmpdir

    in_maps = make_in_maps(**inputs)
    nc = get_nc()
    res = run_bass_kernel_spmd(
        nc, in_maps, core_ids=list(range(NCORES)), trace=True
    )
    return assemble(res.results), res
